# revision 23
# baseline (speedup 1.0000x reference)
"""Trainium2 Bass kernel for nn_DTransformer (sparse decay attention layer).

Sharding: 8 cores = 4 batches x 2 interleaved q-stripes.
  stripe 0 -> q-tiles {0,3,4,7}, stripe 1 -> q-tiles {1,2,5,6} (of 8 tiles
  of 128 rows).  Both stripes have equal causal work (18 k-tile units).

Per-core pipeline (all matmul operands bf16, accumulation fp32):
  - weights/activations transposed via XBAR dma_start_transpose (no PE
    transposes, no PSUM->SBUF copy passes for transposes)
  - decay attention per (q-tile, head):
      S = qk (PSUM) -> e=exp(S) -> cum=scan(e) -> Z=tail
      x = (cum-Z)*negp          (pool STT; negp = static slice of basem)
      u = (x*rz)^0.5            (DVE pow)  [or Ln/Exp path if USE_POW=0]
      f = exp(-|gamma|*u)       (scalar, scale AP)
      s2 = S*f (pool), e2 = exp(s2) bf16 + accum Z2 (scalar)
      maxout: cc = min(Z2/m2,5)/Z2 -> p = e2*cc (DVE STT, bf16)
      pT via dma transpose; out_h = v^T p (PE); concT (scalar copy)
  - output projection + residual + LayerNorm as in the reference.
One activation table (natural_log_exp_and_others) serves Exp/Ln/Copy/Abs:
earlier tables are stripped of Exp/Ln at emission time so the table-load
pass picks the combined table once instead of ping-ponging per call.
"""

import numpy as np

import concourse.bacc as bacc
import concourse.tile as tile
import concourse.bass as bass
from concourse import mybir

P = 128
F32 = mybir.dt.float32
BF16 = mybir.dt.bfloat16
AF = mybir.ActivationFunctionType
ALU = mybir.AluOpType
NEG = -1.0e30

B, T, D, H = 4, 1024, 1024, 16
QTILES_A = [0, 3, 4, 7]
QTILES_B = [1, 2, 5, 6]
USE_POW = False  # ALU pow is not a valid DVE tensor_scalar op on TRN2

# ---- activation-table pinning -------------------------------------------
# bacc's insert_act_table_loads picks, per activation, the first table that
# serves the function; Exp and Ln live in different first-match tables which
# makes it reload 1.3us tables per call.  Strip Exp/Ln from every table
# except the combined one so both resolve to natural_log_exp_and_others.
import concourse.hw_specs as _hw_specs

_orig_tables = _hw_specs.get_activation_tables


def _patched_tables(arch):
    out = {}
    for name, s in _orig_tables(arch).items():
        if name != "natural_log_exp_and_others":
            s = s - {AF.Exp, AF.Ln}
        out[name] = s
    return out


bacc.get_activation_tables = _patched_tables


def emit(tc, io, qtiles, T=T, D=D, H=H):
    nc = tc.nc
    dk = D // H                  # 64
    DT = D // P                  # 8 contraction tiles
    ET = D // P                  # 8 e tiles
    NQ = len(qtiles)
    TQ = NQ * P
    TK = (max(qtiles) + 1) * P   # k extent needed
    TTV = max(qtiles) + 1        # v tiles needed
    HPT = P // dk                # heads per e-tile (2)
    eps = 1e-5
    # stage split: low q-tiles run attention between the two projection
    # stages so the ACT-bound attention overlaps the PE-bound projections
    q_low, q_high = list(enumerate(qtiles))[:2], list(enumerate(qtiles))[2:]
    K1 = qtiles[1] + 1           # k/v tiles needed by stage-2 attention

    xq, xk, xv = io["xq"], io["xk"], io["xv"]
    wq, wv, wo = io["wq"], io["wv"], io["wo"]
    bq, bv, bo = io["bq"], io["bv"], io["bo"]
    gam, lng, lnb = io["gam"], io["lng"], io["lnb"]
    y = io["y"]

    from contextlib import ExitStack
    _stack = ExitStack()
    cpool = _stack.enter_context(tc.tile_pool(name="consts", bufs=1))
    ppool = _stack.enter_context(tc.tile_pool(name="persist", bufs=1))

    # ---- constants ----
    from concourse.masks import make_identity
    ident32 = cpool.tile([P, P], F32)
    make_identity(nc, ident32)
    ident16 = cpool.tile([P, P], BF16)
    nc.vector.tensor_copy(out=ident16, in_=ident32)
    # additive causal masks, one per q-tile, covering the last 512-chunk
    masks = {}
    mtmp = cpool.tile([P, 512], F32)
    for gi in sorted(set(qtiles)):
        Lk = (gi + 1) * P
        lastc0 = ((Lk - 1) // 512) * 512
        lastw = Lk - lastc0
        moff = gi * P - lastc0
        nc.gpsimd.memset(mtmp[:, :lastw], 0.0)
        nc.gpsimd.affine_select(
            out=mtmp[:, :lastw], in_=mtmp[:, :lastw], compare_op=ALU.is_ge,
            fill=NEG, base=moff - 1, pattern=[[-1, lastw]], channel_multiplier=1,
        )
        mk = cpool.tile([P, lastw], BF16, tag=f"mask{gi}")
        nc.vector.tensor_copy(out=mk, in_=mtmp[:, :lastw])
        masks[gi] = (lastc0, lastw, mk)
    # basem[p, c] = min(c - 1024 - p, 0); negp for q-tile gi starts at col
    # 1024 - gi*128
    basem = cpool.tile([P, 2 * T], F32)
    nc.gpsimd.iota(basem, pattern=[[1, 2 * T]], base=-T,
                   channel_multiplier=-1, allow_small_or_imprecise_dtypes=True)
    nc.vector.tensor_scalar_min(basem, basem, 0.0)

    ones16 = cpool.tile([1, 512], BF16)
    nc.vector.memset(ones16, 1.0)
    bstage = cpool.tile([1, D], F32)
    bq16 = cpool.tile([1, D], BF16)
    bv16 = cpool.tile([1, D], BF16)
    bo16 = cpool.tile([1, D], BF16)
    for bsrc, b16 in ((bq, bq16), (bv, bv16), (bo, bo16)):
        nc.sync.dma_start(out=bstage, in_=bsrc[None, :])
        nc.vector.tensor_copy(out=b16, in_=bstage)
    gstage = cpool.tile([P, H], F32)
    nc.sync.dma_start(out=gstage, in_=bass.AP(tensor=gam.tensor, offset=gam.offset,
                                              ap=[[0, P]] + gam.ap))
    gabs = cpool.tile([P, H], F32)
    nc.scalar.activation(out=gabs, in_=gstage, func=AF.Abs)
    lngam = cpool.tile([P, H], F32)      # ln|gamma|
    nc.scalar.activation(out=lngam, in_=gabs, func=AF.Ln)
    eps_col = cpool.tile([P, 1], F32)
    nc.vector.memset(eps_col, eps)

    # ---- persistent tensors ----
    kT = ppool.tile([P, ET, TK], BF16, tag="kT")      # kT[e', et, t]
    qT = ppool.tile([P, ET, TQ], BF16, tag="qT")      # scaled by 1/8
    vb = ppool.tile([P, TTV, D], BF16, tag="v")       # v[t', tt, e]
    qnat = ppool.tile([P, NQ, D], BF16, tag="qnat")   # residual (bf16)
    woT = ppool.tile([P, ET, D], BF16, tag="woT")     # woT[e', et, d]
    concT = ppool.tile([P, ET, TQ], BF16, tag="concT")

    # ---- shared pools (projection + attention interleave) ----
    _stage = ExitStack()
    wtmp = _stage.enter_context(tc.tile_pool(name="wtmp", bufs=2))
    wtb = _stage.enter_context(tc.tile_pool(name="wtb", bufs=1))
    xkb = _stage.enter_context(tc.tile_pool(name="xkb", bufs=1))
    xtb = _stage.enter_context(tc.tile_pool(name="xtb", bufs=2))
    wpsum = _stage.enter_context(tc.tile_pool(name="wpsum", bufs=2, space="PSUM"))
    atmp = _stage.enter_context(tc.tile_pool(name="atmp", bufs=3))
    acum = _stage.enter_context(tc.tile_pool(name="acum", bufs=2))
    apair = _stage.enter_context(tc.tile_pool(name="apair", bufs=2))
    aptp = _stage.enter_context(tc.tile_pool(name="aptp", bufs=1))
    amini = _stage.enter_context(tc.tile_pool(name="amini", bufs=4))
    spsum = _stage.enter_context(tc.tile_pool(name="spsum", bufs=2, space="PSUM"))
    vpsum = _stage.enter_context(tc.tile_pool(name="vpsum", bufs=2, space="PSUM"))

    def load_row16(src_ap):
        """DMA a [P, D] fp32 row tile and cast to bf16 (DVE)."""
        r32_ = wtmp.tile([P, D], F32, tag="row32")
        nc.sync.dma_start(out=r32_, in_=src_ap)
        r16 = wtmp.tile([P, D], BF16, tag="row16")
        nc.vector.tensor_copy(out=r16, in_=r32_)
        return r16

    def load_wT(wsrc, dst):
        """weight (rows, D) -> dst[p, m, row] = w[row, m*128+p] (bf16)"""
        for rt in range(ET):
            r16 = load_row16(wsrc[rt * P:(rt + 1) * P, :])
            nc.sync.dma_start_transpose(out=dst[:, :, rt * P:(rt + 1) * P],
                                        in_=r16)

    def k_proj(wqT, th, tw):
        """kT[:, et, th:th+tw] for one 512-wide chunk of k positions."""
        xkC = xkb.tile([P, DT, 512], BF16, tag="xkC")
        for i in range(tw // P):
            tt = th // P + i
            r16 = load_row16(xk[tt * P:(tt + 1) * P, :])
            nc.sync.dma_start_transpose(out=xkC[:, :, i * P:(i + 1) * P], in_=r16)
        for et in range(ET):
            ps_ = wpsum.tile([P, 512], F32, tag="proj")
            for dc in range(DT):
                nc.tensor.matmul(ps_[:, :tw], wqT[:, dc, et * P:(et + 1) * P],
                                 xkC[:, dc, :tw], start=(dc == 0), stop=False)
            nc.tensor.matmul(ps_[:, :tw], bq16[:, et * P:(et + 1) * P],
                             ones16[:, :tw], start=False, stop=True)
            nc.vector.tensor_copy(out=kT[:, et, th:th + tw], in_=ps_[:, :tw])

    def q_proj(wqT):
        for j in range(NQ):
            r32_ = wtmp.tile([P, D], F32, tag="row32")
            nc.sync.dma_start(out=r32_, in_=xq[j * P:(j + 1) * P, :])
            nc.vector.tensor_copy(out=qnat[:, j, :], in_=r32_)
            xt = xtb.tile([P, DT, P], BF16, tag="xt128")
            nc.sync.dma_start_transpose(out=xt, in_=qnat[:, j, :])
            for et in range(ET):
                ps_ = wpsum.tile([P, 512], F32, tag="proj")
                for dc in range(DT):
                    nc.tensor.matmul(ps_[:, :P], wqT[:, dc, et * P:(et + 1) * P],
                                     xt[:, dc, :], start=(dc == 0), stop=False)
                nc.tensor.matmul(ps_[:, :P], bq16[:, et * P:(et + 1) * P],
                                 ones16[:, :P], start=False, stop=True)
                nc.vector.tensor_scalar(out=qT[:, et, j * P:(j + 1) * P],
                                        in0=ps_[:, :P],
                                        scalar1=1.0 / float(np.sqrt(dk)),
                                        scalar2=None, op0=ALU.mult)

    def v_proj(wvT, t0, t1):
        for tt in range(t0, t1):
            r16 = load_row16(xv[tt * P:(tt + 1) * P, :])
            xt = xtb.tile([P, DT, P], BF16, tag="xt128")
            nc.sync.dma_start_transpose(out=xt, in_=r16)
            for fh in range(0, D, 512):
                ps_ = wpsum.tile([P, 512], F32, tag="proj")
                for dc in range(DT):
                    nc.tensor.matmul(ps_, xt[:, dc, :], wvT[:, dc, fh:fh + 512],
                                     start=(dc == 0), stop=False)
                nc.tensor.matmul(ps_, ones16[:, :P], bv16[:, fh:fh + 512],
                                 start=False, stop=True)
                nc.vector.tensor_copy(out=vb[:, tt, fh:fh + 512], in_=ps_)

    def attn_qtile(j, gi):
        Lk = (gi + 1) * P
        nkt = gi + 1
        negp = basem[:, T - gi * P:T - gi * P + Lk]
        lastc0, lastw, mk = masks[gi]

        for hp in range(H // 2):
            e2p = apair.tile([P, 2, Lk], BF16, tag=f"e2_{j}")
            Z2p = amini.tile([P, 2], F32, tag="Z2p")
            Zp = amini.tile([P, 2], F32, tag="Zp")
            lnZp = amini.tile([P, 2], F32, tag="lnZp")
            lnagz = amini.tile([P, 2], F32, tag="lnagz")
            S_pair, cum_pair = [], []
            for hh in range(2):
                h = 2 * hp + hh
                et, po = h // HPT, (h % HPT) * dk
                S = spsum.tile([P, T], F32, tag="S")
                S_pair.append(S)
                for c0 in range(0, Lk, 512):
                    w_ = min(512, Lk - c0)
                    nc.tensor.matmul(S[:, c0:c0 + w_],
                                     qT[po:po + dk, et, j * P:(j + 1) * P],
                                     kT[po:po + dk, et, c0:c0 + w_],
                                     start=True, stop=(c0 != lastc0))
                nc.tensor.matmul(S[:, lastc0:Lk], ident16, mk,
                                 start=False, stop=True)
                e_ = atmp.tile([P, T], F32, tag="e")
                nc.scalar.activation(out=e_[:, :Lk], in_=S[:, :Lk], func=AF.Exp)
                cum = acum.tile([P, T], F32, tag="cum")
                cum_pair.append(cum)
                nc.vector.tensor_tensor_scan(
                    out=cum[:, :Lk], data0=e_[:, :Lk], data1=e_[:, :Lk],
                    initial=0.0, op0=ALU.add, op1=ALU.bypass)
                nc.vector.tensor_copy(out=Zp[:, hh:hh + 1], in_=cum[:, Lk - 1:Lk])
                if gi == 0:
                    nc.vector.memset(Zp[0:1, hh:hh + 1], 1.0)
                # x = (cum - Z) * negp  (pool, in-place on cum then into e_)
                nc.gpsimd.tensor_tensor(out=cum[:, :Lk], in0=cum[:, :Lk],
                                        in1=Zp[:, hh:hh + 1].to_broadcast([P, Lk]),
                                        op=ALU.subtract)
                nc.gpsimd.tensor_tensor(out=e_[:, :Lk], in0=cum[:, :Lk],
                                        in1=negp, op=ALU.mult)
                cum_pair[hh] = (e_, cum)
            # pair-wide: ln Z and the dist/gamma bias ln|g| - 0.5 ln Z
            nc.scalar.activation(out=lnZp, in_=Zp, func=AF.Ln)
            nc.vector.tensor_scalar(out=lnagz, in0=lnZp, scalar1=-0.5,
                                    scalar2=None, op0=ALU.mult)
            nc.vector.tensor_tensor(out=lnagz, in0=lnagz,
                                    in1=lngam[:, 2 * hp:2 * hp + 2], op=ALU.add)
            for hh in range(2):
                e_, cum = cum_pair[hh]
                S = S_pair[hh]
                # f = exp(-exp(0.5 ln x + ln|g| - 0.5 ln Z))
                nc.scalar.activation(out=e_[:, :Lk], in_=e_[:, :Lk], func=AF.Ln)
                nc.scalar.activation(out=e_[:, :Lk], in_=e_[:, :Lk],
                                     func=AF.Exp, scale=0.5,
                                     bias=lnagz[:, hh:hh + 1])
                nc.scalar.activation(out=e_[:, :Lk], in_=e_[:, :Lk],
                                     func=AF.Exp, scale=-1.0)
                # s2 = S * f; e2 = exp(s2) (bf16) + row sum
                nc.vector.tensor_tensor(out=cum[:, :Lk], in0=S[:, :Lk],
                                        in1=e_[:, :Lk], op=ALU.mult)
                nc.scalar.activation(out=e2p[:, hh, :], in_=cum[:, :Lk],
                                     func=AF.Exp, accum_out=Z2p[:, hh:hh + 1])
            # maxout scale: cc = min(Z2/m2, 5) / Z2
            m2p = amini.tile([P, 2], F32, tag="m2p")
            nc.vector.tensor_reduce(out=m2p, in_=e2p,
                                    axis=mybir.AxisListType.X, op=ALU.max)
            rz2p = amini.tile([P, 2], F32, tag="rz2p")
            nc.vector.reciprocal(out=rz2p, in_=Z2p)
            rmp = amini.tile([P, 2], F32, tag="rmp")
            nc.vector.reciprocal(out=rmp, in_=m2p)
            scp = amini.tile([P, 2], F32, tag="scp")
            nc.vector.tensor_tensor(out=scp, in0=Z2p, in1=rmp, op=ALU.mult)
            nc.vector.tensor_scalar_min(scp, scp, 5.0)
            ccp = amini.tile([P, 2], F32, tag="ccp")
            nc.vector.tensor_tensor(out=ccp, in0=scp, in1=rz2p, op=ALU.mult)
            # p = e2 * cc in-place (scalar Copy+scale, table-free)
            for hh in range(2):
                nc.scalar.mul(out=e2p[:, hh, :], in_=e2p[:, hh, :],
                              mul=ccp[:, hh:hh + 1])
            if gi == 0:
                nc.vector.memset(e2p[0:1, :, :], 0.0)
            # pair transpose: pT[k', hh*nkt + kt, q]
            pT = aptp.tile([P, 2 * nkt, P], BF16, tag=f"pT_{j}")
            nc.sync.dma_start_transpose(out=pT, in_=e2p)
            # PV for both heads into one [P, 128] PSUM tile
            ovp = vpsum.tile([P, P], F32, tag="ovp")
            for hh in range(2):
                h = 2 * hp + hh
                po = (h % HPT) * dk
                for kt in range(nkt):
                    nc.tensor.matmul(ovp[po:po + dk, :],
                                     vb[:, kt, h * dk:(h + 1) * dk],
                                     pT[:, hh * nkt + kt, :],
                                     start=(kt == 0), stop=(kt == nkt - 1))
            nc.vector.tensor_copy(out=concT[:, hp, j * P:(j + 1) * P], in_=ovp)

    # ---------------- stage 1: low-extent projections ----------------
    wqT = wtb.tile([P, DT, D], BF16, tag="wqT")
    load_wT(wq, wqT)
    k_proj(wqT, 0, min(512, TK))
    q_proj(wqT)
    wvT = wtb.tile([P, DT, D], BF16, tag="wvT")
    load_wT(wv, wvT)
    v_proj(wvT, 0, K1)

    # ---------------- stage 2: attention on low q-tiles ----------------
    for j, gi in q_low:
        attn_qtile(j, gi)

    # ---------------- stage 3: remaining projections ----------------
    if TK > 512:
        k_proj(wqT, 512, TK - 512)
    v_proj(wvT, K1, TTV)
    load_wT(wo, woT)

    # ---------------- stage 4: attention on high q-tiles ----------------
    for j, gi in q_high:
        attn_qtile(j, gi)

    _stage.close()

    # ---------------- output projection + residual + layernorm ----------------
    with tc.tile_pool(name="otmp", bufs=2) as otmp, \
         tc.tile_pool(name="omini", bufs=2) as omini, \
         tc.tile_pool(name="opsum", bufs=2, space="PSUM") as opsum:
        lng_bc = otmp.tile([P, D], F32, tag="lng")
        nc.sync.dma_start(out=lng_bc, in_=bass.AP(tensor=lng.tensor, offset=lng.offset,
                                                  ap=[[0, P]] + lng.ap))
        lnb_bc = otmp.tile([P, D], F32, tag="lnb")
        nc.sync.dma_start(out=lnb_bc, in_=bass.AP(tensor=lnb.tensor, offset=lnb.offset,
                                                  ap=[[0, P]] + lnb.ap))
        for j in range(NQ):
            xsb = otmp.tile([P, D], F32, tag="xsb")
            for fh in range(0, D, 512):
                ps_ = opsum.tile([P, 512], F32, tag="attn")
                for et in range(ET):
                    nc.tensor.matmul(ps_, concT[:, et, j * P:(j + 1) * P],
                                     woT[:, et, fh:fh + 512],
                                     start=(et == 0), stop=False)
                nc.tensor.matmul(ps_, ones16[:, :P], bo16[:, fh:fh + 512],
                                 start=False, stop=True)
                nc.vector.tensor_tensor(out=xsb[:, fh:fh + 512], in0=ps_,
                                        in1=qnat[:, j, fh:fh + 512], op=ALU.add)
            stats = omini.tile([P, 2, 6], F32, tag="stats")
            for sg in range(2):
                nc.vector.bn_stats(out=stats[:, sg, :],
                                   in_=xsb[:, sg * 512:(sg + 1) * 512])
            mv = omini.tile([P, 2], F32, tag="mv")
            nc.vector.bn_aggr(out=mv, in_=stats)
            rstd = omini.tile([P, 1], F32, tag="rstd")
            nc.scalar.activation(out=rstd, in_=mv[:, 1:2], func=AF.Ln, bias=eps_col)
            nc.scalar.activation(out=rstd, in_=rstd, func=AF.Exp, scale=-0.5)
            nmr = omini.tile([P, 1], F32, tag="nmr")
            nc.vector.tensor_tensor(out=nmr, in0=mv[:, 0:1], in1=rstd, op=ALU.mult)
            nc.vector.tensor_scalar_mul(nmr, nmr, -1.0)
            ysb = otmp.tile([P, D], F32, tag="ysb")
            nc.scalar.activation(out=ysb, in_=xsb, func=AF.Identity,
                                 bias=nmr, scale=rstd)
            nc.vector.tensor_tensor(out=ysb, in0=ysb, in1=lng_bc, op=ALU.mult)
            nc.vector.tensor_tensor(out=ysb, in0=ysb, in1=lnb_bc, op=ALU.add)
            nc.sync.dma_start(out=y[j * P:(j + 1) * P, :], in_=ysb)

    _stack.close()


# ------------------------------------------------------------------
# program build + host-side runner
# ------------------------------------------------------------------

def build_program(qtiles, T=T, D=D, H=H):
    NQ = len(qtiles)
    nc = bacc.Bacc("TRN2", target_bir_lowering=False, debug=False, num_devices=4)
    io = {}
    io["xq"] = nc.dram_tensor("xq", [NQ * P, D], F32, kind="ExternalInput").ap()
    io["xk"] = nc.dram_tensor("xk", [T, D], F32, kind="ExternalInput").ap()
    io["xv"] = nc.dram_tensor("xv", [T, D], F32, kind="ExternalInput").ap()
    io["wq"] = nc.dram_tensor("wq", [D, D], F32, kind="ExternalInput").ap()
    io["wv"] = nc.dram_tensor("wv", [D, D], F32, kind="ExternalInput").ap()
    io["wo"] = nc.dram_tensor("wo", [D, D], F32, kind="ExternalInput").ap()
    io["bq"] = nc.dram_tensor("bq", [D], F32, kind="ExternalInput").ap()
    io["bv"] = nc.dram_tensor("bv", [D], F32, kind="ExternalInput").ap()
    io["bo"] = nc.dram_tensor("bo", [D], F32, kind="ExternalInput").ap()
    io["gam"] = nc.dram_tensor("gam", [H], F32, kind="ExternalInput").ap()
    io["lng"] = nc.dram_tensor("lng", [D], F32, kind="ExternalInput").ap()
    io["lnb"] = nc.dram_tensor("lnb", [D], F32, kind="ExternalInput").ap()
    io["y"] = nc.dram_tensor("y", [NQ * P, D], F32, kind="ExternalOutput").ap()
    with tile.TileContext(nc) as tc:
        emit(tc, io, qtiles, T=T, D=D, H=H)
    nc.compile()
    return nc


def make_in_maps(inputs, qtiles):
    """Per-core input dicts for one stripe (4 cores, batches 0..3)."""
    q = np.asarray(inputs["query"], np.float32)
    k = np.asarray(inputs["key"], np.float32)
    v = np.asarray(inputs["values"], np.float32)
    rows = np.concatenate([np.arange(g * P, (g + 1) * P) for g in qtiles])
    shared = {
        "wq": np.ascontiguousarray(inputs["Wq"], np.float32),
        "wv": np.ascontiguousarray(inputs["Wv"], np.float32),
        "wo": np.ascontiguousarray(inputs["Wo"], np.float32),
        "bq": np.ascontiguousarray(inputs["bq"], np.float32),
        "bv": np.ascontiguousarray(inputs["bv"], np.float32),
        "bo": np.ascontiguousarray(inputs["bo"], np.float32),
        "gam": np.ascontiguousarray(inputs["gammas"], np.float32),
        "lng": np.ascontiguousarray(inputs["ln_g"], np.float32),
        "lnb": np.ascontiguousarray(inputs["ln_b"], np.float32),
    }
    maps = []
    for b in range(B):
        m = dict(shared)
        m["xq"] = np.ascontiguousarray(q[b][rows])
        m["xk"] = np.ascontiguousarray(k[b])
        m["xv"] = np.ascontiguousarray(v[b])
        maps.append(m)
    return maps


class _Runner:
    """Runs the two stripe programs concurrently on devices 0-3 / 4-7."""

    def __init__(self):
        self.nc_a = build_program(QTILES_A)
        self.nc_b = build_program(QTILES_B)
        self._fns = None

    def _make_fn(self, nc, devices):
        import jax
        from jax.sharding import Mesh, PartitionSpec
        from jax.experimental.shard_map import shard_map
        from concourse import bass2jax
        from concourse.bass2jax import _bass_exec_p, partition_id_tensor

        bass2jax.install_neuronx_cc_hook()
        partition_name = (nc.partition_id_tensor.name
                          if nc.partition_id_tensor else None)
        in_names, out_names, out_avals, zero_outs = [], [], [], []
        for alloc in nc.m.functions[0].allocations:
            if not isinstance(alloc, mybir.MemoryLocationSet):
                continue
            name = alloc.memorylocations[0].name
            if alloc.kind == "ExternalInput":
                if name != partition_name:
                    in_names.append(name)
            elif alloc.kind == "ExternalOutput":
                shape = tuple(alloc.tensor_shape)
                dtype = mybir.dt.np(alloc.dtype)
                out_names.append(name)
                out_avals.append(jax.core.ShapedArray(shape, dtype))
                zero_outs.append(np.zeros(shape, dtype))
        n_params = len(in_names)
        all_in = list(in_names) + list(out_names)
        if partition_name is not None:
            all_in.append(partition_name)

        def _body(*args):
            operands = list(args)
            if partition_name is not None:
                operands.append(partition_id_tensor())
            outs = _bass_exec_p.bind(
                *operands, out_avals=tuple(out_avals), in_names=tuple(all_in),
                out_names=tuple(out_names), lowering_input_output_aliases=(),
                sim_require_finite=True, sim_require_nnan=True, nc=nc)
            return tuple(outs)

        mesh = Mesh(np.asarray(devices), ("core",))
        n = n_params + len(out_names)
        fn = jax.jit(shard_map(_body, mesh=mesh,
                               in_specs=(PartitionSpec("core"),) * n,
                               out_specs=(PartitionSpec("core"),) * len(out_names),
                               check_rep=False),
                     keep_unused=True)
        return fn, in_names, out_names, zero_outs

    def fns(self):
        if self._fns is None:
            import jax
            devs = jax.devices()
            self._fns = (self._make_fn(self.nc_a, devs[0:4]),
                         self._make_fn(self.nc_b, devs[4:8]))
        return self._fns

    def _concat_args(self, spec, in_maps):
        fn, in_names, out_names, zero_outs = spec
        args = [np.concatenate([np.asarray(m[nm]) for m in in_maps], axis=0)
                for nm in in_names]
        args += [np.zeros((4 * z.shape[0], *z.shape[1:]), z.dtype) for z in zero_outs]
        return args

    def run(self, inputs):
        import jax
        spec_a, spec_b = self.fns()
        maps_a = make_in_maps(inputs, QTILES_A)
        maps_b = make_in_maps(inputs, QTILES_B)
        oa = spec_a[0](*self._concat_args(spec_a, maps_a))
        ob = spec_b[0](*self._concat_args(spec_b, maps_b))
        jax.block_until_ready((oa, ob))
        ya = np.asarray(oa[0]).reshape(4, len(QTILES_A) * P, D)
        yb = np.asarray(ob[0]).reshape(4, len(QTILES_B) * P, D)
        out = np.empty((B, T, D), np.float32)
        for b in range(B):
            for jj, g in enumerate(QTILES_A):
                out[b, g * P:(g + 1) * P] = ya[b, jj * P:(jj + 1) * P]
            for jj, g in enumerate(QTILES_B):
                out[b, g * P:(g + 1) * P] = yb[b, jj * P:(jj + 1) * P]
        return out


_runner = None


def kernel(**inputs) -> np.ndarray:
    global _runner
    if _runner is None:
        _runner = _Runner()
    return _runner.run(inputs)


# revision 25
# speedup vs baseline: 1.0197x; 1.0197x over previous
"""Trainium2 Bass kernel for nn_DTransformer (sparse decay attention layer).

Sharding: 8 cores = 4 batches x 2 interleaved q-stripes.
  stripe 0 -> q-tiles {0,3,4,7}, stripe 1 -> q-tiles {1,2,5,6} (of 8 tiles
  of 128 rows).  Both stripes have equal causal work (18 k-tile units).

Per-core pipeline (all matmul operands bf16, accumulation fp32):
  - weights/activations transposed via XBAR dma_start_transpose (no PE
    transposes, no PSUM->SBUF copy passes for transposes)
  - decay attention per (q-tile, head):
      S = qk (PSUM) -> e=exp(S) -> cum=scan(e) -> Z=tail
      x = (cum-Z)*negp          (pool STT; negp = static slice of basem)
      u = (x*rz)^0.5            (DVE pow)  [or Ln/Exp path if USE_POW=0]
      f = exp(-|gamma|*u)       (scalar, scale AP)
      s2 = S*f (pool), e2 = exp(s2) bf16 + accum Z2 (scalar)
      maxout: cc = min(Z2/m2,5)/Z2 -> p = e2*cc (DVE STT, bf16)
      pT via dma transpose; out_h = v^T p (PE); concT (scalar copy)
  - output projection + residual + LayerNorm as in the reference.
One activation table (natural_log_exp_and_others) serves Exp/Ln/Copy/Abs:
earlier tables are stripped of Exp/Ln at emission time so the table-load
pass picks the combined table once instead of ping-ponging per call.
"""

import numpy as np

import concourse.bacc as bacc
import concourse.tile as tile
import concourse.bass as bass
from concourse import mybir

P = 128
F32 = mybir.dt.float32
BF16 = mybir.dt.bfloat16
AF = mybir.ActivationFunctionType
ALU = mybir.AluOpType
NEG = -1.0e30

B, T, D, H = 4, 1024, 1024, 16
QTILES_A = [0, 3, 4, 7]
QTILES_B = [1, 2, 5, 6]
USE_POW = False  # ALU pow is not a valid DVE tensor_scalar op on TRN2

# ---- activation-table pinning -------------------------------------------
# bacc's insert_act_table_loads picks, per activation, the first table that
# serves the function; Exp and Ln live in different first-match tables which
# makes it reload 1.3us tables per call.  Strip Exp/Ln from every table
# except the combined one so both resolve to natural_log_exp_and_others.
import concourse.hw_specs as _hw_specs

_orig_tables = _hw_specs.get_activation_tables


def _patched_tables(arch):
    out = {}
    for name, s in _orig_tables(arch).items():
        if name != "natural_log_exp_and_others":
            s = s - {AF.Exp, AF.Ln}
        out[name] = s
    return out


bacc.get_activation_tables = _patched_tables


def emit(tc, io, qtiles, T=T, D=D, H=H):
    nc = tc.nc
    dk = D // H                  # 64
    DT = D // P                  # 8 contraction tiles
    ET = D // P                  # 8 e tiles
    NQ = len(qtiles)
    TQ = NQ * P
    TK = (max(qtiles) + 1) * P   # k extent needed
    TTV = max(qtiles) + 1        # v tiles needed
    HPT = P // dk                # heads per e-tile (2)
    eps = 1e-5
    # stage split: low q-tiles run attention between the two projection
    # stages so the ACT-bound attention overlaps the PE-bound projections
    q_low, q_high = list(enumerate(qtiles))[:2], list(enumerate(qtiles))[2:]
    K1 = qtiles[1] + 1           # k/v tiles needed by stage-2 attention

    xq, xk, xv = io["xq"], io["xk"], io["xv"]
    wq, wv, wo = io["wq"], io["wv"], io["wo"]
    bq, bv, bo = io["bq"], io["bv"], io["bo"]
    gam, lng, lnb = io["gam"], io["lng"], io["lnb"]
    y = io["y"]

    from contextlib import ExitStack
    _stack = ExitStack()
    cpool = _stack.enter_context(tc.tile_pool(name="consts", bufs=1))
    ppool = _stack.enter_context(tc.tile_pool(name="persist", bufs=1))

    # ---- constants ----
    from concourse.masks import make_identity
    ident32 = cpool.tile([P, P], F32)
    make_identity(nc, ident32)
    ident16 = cpool.tile([P, P], BF16)
    nc.vector.tensor_copy(out=ident16, in_=ident32)
    # additive causal masks, one per q-tile, covering the last 512-chunk
    masks = {}
    mtmp = cpool.tile([P, 512], F32)
    for gi in sorted(set(qtiles)):
        Lk = (gi + 1) * P
        lastc0 = ((Lk - 1) // 512) * 512
        lastw = Lk - lastc0
        moff = gi * P - lastc0
        nc.gpsimd.memset(mtmp[:, :lastw], 0.0)
        nc.gpsimd.affine_select(
            out=mtmp[:, :lastw], in_=mtmp[:, :lastw], compare_op=ALU.is_ge,
            fill=NEG, base=moff - 1, pattern=[[-1, lastw]], channel_multiplier=1,
        )
        mk = cpool.tile([P, lastw], BF16, tag=f"mask{gi}")
        nc.vector.tensor_copy(out=mk, in_=mtmp[:, :lastw])
        masks[gi] = (lastc0, lastw, mk)
    # basem[p, c] = min(c - 1024 - p, 0); negp for q-tile gi starts at col
    # 1024 - gi*128
    basem = cpool.tile([P, 2 * T], F32)
    nc.gpsimd.iota(basem, pattern=[[1, 2 * T]], base=-T,
                   channel_multiplier=-1, allow_small_or_imprecise_dtypes=True)
    nc.vector.tensor_scalar_min(basem, basem, 0.0)

    ones16 = cpool.tile([1, 512], BF16)
    nc.vector.memset(ones16, 1.0)
    bstage = cpool.tile([1, D], F32)
    bq16 = cpool.tile([1, D], BF16)
    bv16 = cpool.tile([1, D], BF16)
    bo16 = cpool.tile([1, D], BF16)
    for bsrc, b16 in ((bq, bq16), (bv, bv16), (bo, bo16)):
        nc.sync.dma_start(out=bstage, in_=bsrc[None, :])
        nc.vector.tensor_copy(out=b16, in_=bstage)
    gstage = cpool.tile([P, H], F32)
    nc.sync.dma_start(out=gstage, in_=bass.AP(tensor=gam.tensor, offset=gam.offset,
                                              ap=[[0, P]] + gam.ap))
    gabs = cpool.tile([P, H], F32)
    nc.scalar.activation(out=gabs, in_=gstage, func=AF.Abs)
    lngam = cpool.tile([P, H], F32)      # ln|gamma|
    nc.scalar.activation(out=lngam, in_=gabs, func=AF.Ln)
    eps_col = cpool.tile([P, 1], F32)
    nc.vector.memset(eps_col, eps)

    # ---- persistent tensors ----
    kT = ppool.tile([P, ET, TK], BF16, tag="kT")      # kT[e', et, t]
    qT = ppool.tile([P, ET, TQ], BF16, tag="qT")      # scaled by 1/8
    vb = ppool.tile([P, TTV, D], BF16, tag="v")       # v[t', tt, e]
    qnat = ppool.tile([P, NQ, D], BF16, tag="qnat")   # residual (bf16)
    woT = ppool.tile([P, ET, D], BF16, tag="woT")     # woT[e', et, d]
    concT = ppool.tile([P, ET, TQ], BF16, tag="concT")

    # ---- shared pools (projection + attention interleave) ----
    _stage = ExitStack()
    wtmp = _stage.enter_context(tc.tile_pool(name="wtmp", bufs=2))
    wtb = _stage.enter_context(tc.tile_pool(name="wtb", bufs=1))
    xkb = _stage.enter_context(tc.tile_pool(name="xkb", bufs=1))
    xtb = _stage.enter_context(tc.tile_pool(name="xtb", bufs=1))
    wpsum = _stage.enter_context(tc.tile_pool(name="wpsum", bufs=1, space="PSUM"))
    atmp = _stage.enter_context(tc.tile_pool(name="atmp", bufs=3))
    acum = _stage.enter_context(tc.tile_pool(name="acum", bufs=2))
    apair = _stage.enter_context(tc.tile_pool(name="apair", bufs=2))
    aptp = _stage.enter_context(tc.tile_pool(name="aptp", bufs=2))
    amini = _stage.enter_context(tc.tile_pool(name="amini", bufs=4))
    spsum = _stage.enter_context(tc.tile_pool(name="spsum", bufs=3, space="PSUM"))
    vpsum = _stage.enter_context(tc.tile_pool(name="vpsum", bufs=1, space="PSUM"))

    def load_row16(src_ap):
        """DMA a [P, D] fp32 row tile and cast to bf16 (DVE)."""
        r32_ = wtmp.tile([P, D], F32, tag="row32")
        nc.sync.dma_start(out=r32_, in_=src_ap)
        r16 = wtmp.tile([P, D], BF16, tag="row16")
        nc.vector.tensor_copy(out=r16, in_=r32_)
        return r16

    def load_wT(wsrc, dst):
        """weight (rows, D) -> dst[p, m, row] = w[row, m*128+p] (bf16)"""
        for rt in range(ET):
            r16 = load_row16(wsrc[rt * P:(rt + 1) * P, :])
            nc.sync.dma_start_transpose(out=dst[:, :, rt * P:(rt + 1) * P],
                                        in_=r16)

    def k_proj(wqT, th, tw):
        """kT[:, et, th:th+tw] for one 512-wide chunk of k positions."""
        xkC = xkb.tile([P, DT, 512], BF16, tag="xkC")
        for i in range(tw // P):
            tt = th // P + i
            r16 = load_row16(xk[tt * P:(tt + 1) * P, :])
            nc.sync.dma_start_transpose(out=xkC[:, :, i * P:(i + 1) * P], in_=r16)
        for et in range(ET):
            ps_ = wpsum.tile([P, 512], F32, tag="proj")
            for dc in range(DT):
                nc.tensor.matmul(ps_[:, :tw], wqT[:, dc, et * P:(et + 1) * P],
                                 xkC[:, dc, :tw], start=(dc == 0), stop=False)
            nc.tensor.matmul(ps_[:, :tw], bq16[:, et * P:(et + 1) * P],
                             ones16[:, :tw], start=False, stop=True)
            nc.scalar.copy(out=kT[:, et, th:th + tw], in_=ps_[:, :tw])

    def q_proj(wqT):
        xqC = xkb.tile([P, DT, 512], BF16, tag="xkC")  # reuse k-chunk buffer
        for j in range(NQ):
            r32_ = wtmp.tile([P, D], F32, tag="row32")
            nc.sync.dma_start(out=r32_, in_=xq[j * P:(j + 1) * P, :])
            nc.vector.tensor_copy(out=qnat[:, j, :], in_=r32_)
            nc.sync.dma_start_transpose(out=xqC[:, :, j * P:(j + 1) * P],
                                        in_=qnat[:, j, :])
        for et in range(ET):
            for th in range(0, TQ, 512):
                tw = min(512, TQ - th)
                ps_ = wpsum.tile([P, 512], F32, tag="proj")
                for dc in range(DT):
                    nc.tensor.matmul(ps_[:, :tw], wqT[:, dc, et * P:(et + 1) * P],
                                     xqC[:, dc, th:th + tw],
                                     start=(dc == 0), stop=False)
                nc.tensor.matmul(ps_[:, :tw], bq16[:, et * P:(et + 1) * P],
                                 ones16[:, :tw], start=False, stop=True)
                nc.vector.tensor_scalar(out=qT[:, et, th:th + tw],
                                        in0=ps_[:, :tw],
                                        scalar1=1.0 / float(np.sqrt(dk)),
                                        scalar2=None, op0=ALU.mult)

    def v_proj(wvT, t0, t1):
        for tt in range(t0, t1):
            r16 = load_row16(xv[tt * P:(tt + 1) * P, :])
            xt = xtb.tile([P, DT, P], BF16, tag="xt128")
            nc.sync.dma_start_transpose(out=xt, in_=r16)
            for fh in range(0, D, 512):
                ps_ = wpsum.tile([P, 512], F32, tag="proj")
                for dc in range(DT):
                    nc.tensor.matmul(ps_, xt[:, dc, :], wvT[:, dc, fh:fh + 512],
                                     start=(dc == 0), stop=False)
                nc.tensor.matmul(ps_, ones16[:, :P], bv16[:, fh:fh + 512],
                                 start=False, stop=True)
                nc.vector.tensor_copy(out=vb[:, tt, fh:fh + 512], in_=ps_)

    pending = []  # deferred PV+concT emissions (software pipeline)

    def flush_pending():
        while pending:
            pT_, j_, gi_, hp_ = pending.pop(0)
            nkt_ = gi_ + 1
            ovp = vpsum.tile([P, P], F32, tag="ovp")
            for hh in range(2):
                h = 2 * hp_ + hh
                po = (h % HPT) * dk
                for kt in range(nkt_):
                    nc.tensor.matmul(ovp[po:po + dk, :],
                                     vb[:, kt, h * dk:(h + 1) * dk],
                                     pT_[:, hh * nkt_ + kt, :],
                                     start=(kt == 0), stop=(kt == nkt_ - 1))
            nc.vector.tensor_copy(out=concT[:, hp_, j_ * P:(j_ + 1) * P], in_=ovp)

    def attn_qtile(j, gi):
        Lk = (gi + 1) * P
        nkt = gi + 1
        negp = basem[:, T - gi * P:T - gi * P + Lk]
        lastc0, lastw, mk = masks[gi]

        for hp in range(H // 2):
            e2p = apair.tile([P, 2, Lk], BF16, tag=f"e2_{j}")
            Z2p = amini.tile([P, 2], F32, tag="Z2p")
            Zp = amini.tile([P, 2], F32, tag="Zp")
            lnZp = amini.tile([P, 2], F32, tag="lnZp")
            lnagz = amini.tile([P, 2], F32, tag="lnagz")
            S_pair, cum_pair = [], []
            for hh in range(2):
                h = 2 * hp + hh
                et, po = h // HPT, (h % HPT) * dk
                S = spsum.tile([P, T], F32, tag="S")
                S_pair.append(S)
                for c0 in range(0, Lk, 512):
                    w_ = min(512, Lk - c0)
                    nc.tensor.matmul(S[:, c0:c0 + w_],
                                     qT[po:po + dk, et, j * P:(j + 1) * P],
                                     kT[po:po + dk, et, c0:c0 + w_],
                                     start=True, stop=(c0 != lastc0))
                nc.tensor.matmul(S[:, lastc0:Lk], ident16, mk,
                                 start=False, stop=True)
                if hh == 1:
                    flush_pending()
                e_ = atmp.tile([P, T], F32, tag="e")
                nc.scalar.activation(out=e_[:, :Lk], in_=S[:, :Lk], func=AF.Exp)
                cum = acum.tile([P, T], F32, tag="cum")
                cum_pair.append(cum)
                nc.vector.tensor_tensor_scan(
                    out=cum[:, :Lk], data0=e_[:, :Lk], data1=e_[:, :Lk],
                    initial=0.0, op0=ALU.add, op1=ALU.bypass)
                nc.vector.tensor_copy(out=Zp[:, hh:hh + 1], in_=cum[:, Lk - 1:Lk])
                if gi == 0:
                    nc.vector.memset(Zp[0:1, hh:hh + 1], 1.0)
                # x = (cum - Z) * negp  (pool, in-place on cum then into e_)
                nc.gpsimd.tensor_tensor(out=cum[:, :Lk], in0=cum[:, :Lk],
                                        in1=Zp[:, hh:hh + 1].to_broadcast([P, Lk]),
                                        op=ALU.subtract)
                nc.gpsimd.tensor_tensor(out=e_[:, :Lk], in0=cum[:, :Lk],
                                        in1=negp, op=ALU.mult)
                cum_pair[hh] = (e_, cum)
            # pair-wide: ln Z and the dist/gamma bias ln|g| - 0.5 ln Z
            nc.scalar.activation(out=lnZp, in_=Zp, func=AF.Ln)
            nc.vector.tensor_scalar(out=lnagz, in0=lnZp, scalar1=-0.5,
                                    scalar2=None, op0=ALU.mult)
            nc.vector.tensor_tensor(out=lnagz, in0=lnagz,
                                    in1=lngam[:, 2 * hp:2 * hp + 2], op=ALU.add)
            for hh in range(2):
                e_, cum = cum_pair[hh]
                S = S_pair[hh]
                # f = exp(-exp(0.5 ln x + ln|g| - 0.5 ln Z))
                nc.scalar.activation(out=e_[:, :Lk], in_=e_[:, :Lk], func=AF.Ln)
                nc.scalar.activation(out=e_[:, :Lk], in_=e_[:, :Lk],
                                     func=AF.Exp, scale=0.5,
                                     bias=lnagz[:, hh:hh + 1])
                nc.scalar.activation(out=e_[:, :Lk], in_=e_[:, :Lk],
                                     func=AF.Exp, scale=-1.0)
                # s2 = S * f; e2 = exp(s2) (bf16) + row sum
                nc.vector.tensor_tensor(out=cum[:, :Lk], in0=S[:, :Lk],
                                        in1=e_[:, :Lk], op=ALU.mult)
                nc.scalar.activation(out=e2p[:, hh, :], in_=cum[:, :Lk],
                                     func=AF.Exp, accum_out=Z2p[:, hh:hh + 1])
            # maxout scale: cc = min(Z2/m2, 5) / Z2
            m2p = amini.tile([P, 2], F32, tag="m2p")
            nc.vector.tensor_reduce(out=m2p, in_=e2p,
                                    axis=mybir.AxisListType.X, op=ALU.max)
            rz2p = amini.tile([P, 2], F32, tag="rz2p")
            nc.vector.reciprocal(out=rz2p, in_=Z2p)
            rmp = amini.tile([P, 2], F32, tag="rmp")
            nc.vector.reciprocal(out=rmp, in_=m2p)
            scp = amini.tile([P, 2], F32, tag="scp")
            nc.vector.tensor_tensor(out=scp, in0=Z2p, in1=rmp, op=ALU.mult)
            nc.vector.tensor_scalar_min(scp, scp, 5.0)
            ccp = amini.tile([P, 2], F32, tag="ccp")
            nc.vector.tensor_tensor(out=ccp, in0=scp, in1=rz2p, op=ALU.mult)
            # p = e2 * cc in-place (scalar Copy+scale, table-free)
            for hh in range(2):
                nc.scalar.mul(out=e2p[:, hh, :], in_=e2p[:, hh, :],
                              mul=ccp[:, hh:hh + 1])
            if gi == 0:
                nc.vector.memset(e2p[0:1, :, :], 0.0)
            # pair transpose: pT[k', hh*nkt + kt, q]; PV is deferred until
            # after the NEXT pair's S matmuls so the in-order PE stream does
            # not stall on this pair's dma transpose
            pT = aptp.tile([P, 2 * nkt, P], BF16, tag=f"pT_{j}")
            nc.sync.dma_start_transpose(out=pT, in_=e2p)
            pending.append((pT, j, gi, hp))

    # ---------------- stage 1: low-extent projections ----------------
    wqT = wtb.tile([P, DT, D], BF16, tag="wqT")
    load_wT(wq, wqT)
    k_proj(wqT, 0, min(512, TK))
    q_proj(wqT)
    wvT = wtb.tile([P, DT, D], BF16, tag="wvT")
    load_wT(wv, wvT)
    v_proj(wvT, 0, K1)

    # ---------------- stage 2: attention on low q-tiles ----------------
    for j, gi in q_low:
        attn_qtile(j, gi)

    # ---------------- stage 3: remaining projections ----------------
    if TK > 512:
        k_proj(wqT, 512, TK - 512)
    v_proj(wvT, K1, TTV)
    load_wT(wo, woT)

    # ---------------- stage 4: attention on high q-tiles ----------------
    for j, gi in q_high:
        attn_qtile(j, gi)
    flush_pending()

    _stage.close()

    # ---------------- output projection + residual + layernorm ----------------
    with tc.tile_pool(name="otmp", bufs=2) as otmp, \
         tc.tile_pool(name="omini", bufs=2) as omini, \
         tc.tile_pool(name="opsum", bufs=2, space="PSUM") as opsum:
        lng_bc = otmp.tile([P, D], F32, tag="lng")
        nc.sync.dma_start(out=lng_bc, in_=bass.AP(tensor=lng.tensor, offset=lng.offset,
                                                  ap=[[0, P]] + lng.ap))
        lnb_bc = otmp.tile([P, D], F32, tag="lnb")
        nc.sync.dma_start(out=lnb_bc, in_=bass.AP(tensor=lnb.tensor, offset=lnb.offset,
                                                  ap=[[0, P]] + lnb.ap))
        for j in range(NQ):
            xsb = otmp.tile([P, D], F32, tag="xsb")
            for fh in range(0, D, 512):
                ps_ = opsum.tile([P, 512], F32, tag="attn")
                for et in range(ET):
                    nc.tensor.matmul(ps_, concT[:, et, j * P:(j + 1) * P],
                                     woT[:, et, fh:fh + 512],
                                     start=(et == 0), stop=False)
                nc.tensor.matmul(ps_, ones16[:, :P], bo16[:, fh:fh + 512],
                                 start=False, stop=True)
                nc.vector.tensor_tensor(out=xsb[:, fh:fh + 512], in0=ps_,
                                        in1=qnat[:, j, fh:fh + 512], op=ALU.add)
            stats = omini.tile([P, 2, 6], F32, tag="stats")
            for sg in range(2):
                nc.vector.bn_stats(out=stats[:, sg, :],
                                   in_=xsb[:, sg * 512:(sg + 1) * 512])
            mv = omini.tile([P, 2], F32, tag="mv")
            nc.vector.bn_aggr(out=mv, in_=stats)
            rstd = omini.tile([P, 1], F32, tag="rstd")
            nc.scalar.activation(out=rstd, in_=mv[:, 1:2], func=AF.Ln, bias=eps_col)
            nc.scalar.activation(out=rstd, in_=rstd, func=AF.Exp, scale=-0.5)
            nmr = omini.tile([P, 1], F32, tag="nmr")
            nc.vector.tensor_tensor(out=nmr, in0=mv[:, 0:1], in1=rstd, op=ALU.mult)
            nc.vector.tensor_scalar_mul(nmr, nmr, -1.0)
            ysb = otmp.tile([P, D], F32, tag="ysb")
            nc.scalar.activation(out=ysb, in_=xsb, func=AF.Identity,
                                 bias=nmr, scale=rstd)
            nc.vector.tensor_tensor(out=ysb, in0=ysb, in1=lng_bc, op=ALU.mult)
            nc.vector.tensor_tensor(out=ysb, in0=ysb, in1=lnb_bc, op=ALU.add)
            nc.sync.dma_start(out=y[j * P:(j + 1) * P, :], in_=ysb)

    _stack.close()


# ------------------------------------------------------------------
# program build + host-side runner
# ------------------------------------------------------------------

def build_program(qtiles, T=T, D=D, H=H):
    NQ = len(qtiles)
    nc = bacc.Bacc("TRN2", target_bir_lowering=False, debug=False, num_devices=4)
    io = {}
    io["xq"] = nc.dram_tensor("xq", [NQ * P, D], F32, kind="ExternalInput").ap()
    io["xk"] = nc.dram_tensor("xk", [T, D], F32, kind="ExternalInput").ap()
    io["xv"] = nc.dram_tensor("xv", [T, D], F32, kind="ExternalInput").ap()
    io["wq"] = nc.dram_tensor("wq", [D, D], F32, kind="ExternalInput").ap()
    io["wv"] = nc.dram_tensor("wv", [D, D], F32, kind="ExternalInput").ap()
    io["wo"] = nc.dram_tensor("wo", [D, D], F32, kind="ExternalInput").ap()
    io["bq"] = nc.dram_tensor("bq", [D], F32, kind="ExternalInput").ap()
    io["bv"] = nc.dram_tensor("bv", [D], F32, kind="ExternalInput").ap()
    io["bo"] = nc.dram_tensor("bo", [D], F32, kind="ExternalInput").ap()
    io["gam"] = nc.dram_tensor("gam", [H], F32, kind="ExternalInput").ap()
    io["lng"] = nc.dram_tensor("lng", [D], F32, kind="ExternalInput").ap()
    io["lnb"] = nc.dram_tensor("lnb", [D], F32, kind="ExternalInput").ap()
    io["y"] = nc.dram_tensor("y", [NQ * P, D], F32, kind="ExternalOutput").ap()
    with tile.TileContext(nc) as tc:
        emit(tc, io, qtiles, T=T, D=D, H=H)
    nc.compile()
    return nc


def make_in_maps(inputs, qtiles):
    """Per-core input dicts for one stripe (4 cores, batches 0..3)."""
    q = np.asarray(inputs["query"], np.float32)
    k = np.asarray(inputs["key"], np.float32)
    v = np.asarray(inputs["values"], np.float32)
    rows = np.concatenate([np.arange(g * P, (g + 1) * P) for g in qtiles])
    shared = {
        "wq": np.ascontiguousarray(inputs["Wq"], np.float32),
        "wv": np.ascontiguousarray(inputs["Wv"], np.float32),
        "wo": np.ascontiguousarray(inputs["Wo"], np.float32),
        "bq": np.ascontiguousarray(inputs["bq"], np.float32),
        "bv": np.ascontiguousarray(inputs["bv"], np.float32),
        "bo": np.ascontiguousarray(inputs["bo"], np.float32),
        "gam": np.ascontiguousarray(inputs["gammas"], np.float32),
        "lng": np.ascontiguousarray(inputs["ln_g"], np.float32),
        "lnb": np.ascontiguousarray(inputs["ln_b"], np.float32),
    }
    maps = []
    for b in range(B):
        m = dict(shared)
        m["xq"] = np.ascontiguousarray(q[b][rows])
        m["xk"] = np.ascontiguousarray(k[b])
        m["xv"] = np.ascontiguousarray(v[b])
        maps.append(m)
    return maps


class _Runner:
    """Runs the two stripe programs concurrently on devices 0-3 / 4-7."""

    def __init__(self):
        self.nc_a = build_program(QTILES_A)
        self.nc_b = build_program(QTILES_B)
        self._fns = None

    def _make_fn(self, nc, devices):
        import jax
        from jax.sharding import Mesh, PartitionSpec
        from jax.experimental.shard_map import shard_map
        from concourse import bass2jax
        from concourse.bass2jax import _bass_exec_p, partition_id_tensor

        bass2jax.install_neuronx_cc_hook()
        partition_name = (nc.partition_id_tensor.name
                          if nc.partition_id_tensor else None)
        in_names, out_names, out_avals, zero_outs = [], [], [], []
        for alloc in nc.m.functions[0].allocations:
            if not isinstance(alloc, mybir.MemoryLocationSet):
                continue
            name = alloc.memorylocations[0].name
            if alloc.kind == "ExternalInput":
                if name != partition_name:
                    in_names.append(name)
            elif alloc.kind == "ExternalOutput":
                shape = tuple(alloc.tensor_shape)
                dtype = mybir.dt.np(alloc.dtype)
                out_names.append(name)
                out_avals.append(jax.core.ShapedArray(shape, dtype))
                zero_outs.append(np.zeros(shape, dtype))
        n_params = len(in_names)
        all_in = list(in_names) + list(out_names)
        if partition_name is not None:
            all_in.append(partition_name)

        def _body(*args):
            operands = list(args)
            if partition_name is not None:
                operands.append(partition_id_tensor())
            outs = _bass_exec_p.bind(
                *operands, out_avals=tuple(out_avals), in_names=tuple(all_in),
                out_names=tuple(out_names), lowering_input_output_aliases=(),
                sim_require_finite=True, sim_require_nnan=True, nc=nc)
            return tuple(outs)

        mesh = Mesh(np.asarray(devices), ("core",))
        n = n_params + len(out_names)
        fn = jax.jit(shard_map(_body, mesh=mesh,
                               in_specs=(PartitionSpec("core"),) * n,
                               out_specs=(PartitionSpec("core"),) * len(out_names),
                               check_rep=False),
                     keep_unused=True)
        return fn, in_names, out_names, zero_outs

    def fns(self):
        if self._fns is None:
            import jax
            devs = jax.devices()
            self._fns = (self._make_fn(self.nc_a, devs[0:4]),
                         self._make_fn(self.nc_b, devs[4:8]))
        return self._fns

    def _concat_args(self, spec, in_maps):
        fn, in_names, out_names, zero_outs = spec
        args = [np.concatenate([np.asarray(m[nm]) for m in in_maps], axis=0)
                for nm in in_names]
        args += [np.zeros((4 * z.shape[0], *z.shape[1:]), z.dtype) for z in zero_outs]
        return args

    def run(self, inputs):
        import jax
        spec_a, spec_b = self.fns()
        maps_a = make_in_maps(inputs, QTILES_A)
        maps_b = make_in_maps(inputs, QTILES_B)
        oa = spec_a[0](*self._concat_args(spec_a, maps_a))
        ob = spec_b[0](*self._concat_args(spec_b, maps_b))
        jax.block_until_ready((oa, ob))
        ya = np.asarray(oa[0]).reshape(4, len(QTILES_A) * P, D)
        yb = np.asarray(ob[0]).reshape(4, len(QTILES_B) * P, D)
        out = np.empty((B, T, D), np.float32)
        for b in range(B):
            for jj, g in enumerate(QTILES_A):
                out[b, g * P:(g + 1) * P] = ya[b, jj * P:(jj + 1) * P]
            for jj, g in enumerate(QTILES_B):
                out[b, g * P:(g + 1) * P] = yb[b, jj * P:(jj + 1) * P]
        return out


_runner = None


def kernel(**inputs) -> np.ndarray:
    global _runner
    if _runner is None:
        _runner = _Runner()
    return _runner.run(inputs)
